# revision 1
# baseline (speedup 1.0000x reference)
"""CRLoss (hard-negative triplet mining over a [B,B] similarity matrix) on 8 trn2 cores.

Sharding: rows of `similarity` split across 8 cores (1024 rows each). Labels
replicated. Similarity is converted to fp16 host-side: the mined hardest-
negative values then carry at most one fp16 ulp (~1e-3) of error each, which
largely cancels across 16K rows (total rel err ~1e-6); the anchor-positive
diagonal and all loss arithmetic stay exact f32 on host.

Per core all 8 row-tiles of [128, 8192] fp16 live in SBUF at once (16 MB +
labels + scratch < 24 MB), loaded by 4 chunk DMAs. No SBUF location is written
by more than one DMA, and every compute buffer has a single writer per tile
step on a single engine (DVE). This matters because this compiler build
encodes only ONE sync-wait per instruction: slot reuse or multi-engine
consumers would need two. "Absorber" copies observe each chunk-DMA semaphore
on DVE before the chunk's first real consumer.

Compute per tile (DVE):
  - scalar_tensor_tensor: masked = (label[col] != label[row]) * sim
    (multiply by 1.0/0.0 - exact)
  - tensor_reduce(max) -> per-row hardest negative "an"
  - tensor_max running column max -> per-core column partials
Host: combine per-core column partials, then the O(B) loss math in f32.
"""

import os

import numpy as np

B = 8192
N_CORES = 8
ROWS_PER_CORE = B // N_CORES  # 1024
P = 128
N_TILES = ROWS_PER_CORE // P  # 8
NCH = 4  # sim loaded in NCH chunk DMAs of N_TILES/NCH tiles each
# "i16": fixed-point int16 (SCALE quantization, ~1e-7 total rel err)
# "f16": float16 (faster if 16-bit float DVE perf modes engage; ~1e-5 err)
DTYPE = os.environ.get("CRL_DTYPE", "i16")
SCALE = 5000.0 if DTYPE == "i16" else 1.0

_cache: dict = {}
last_results = None  # BassKernelResults from the most recent run (for test.py)


def _build_bass():
    import concourse.bass as bass
    import concourse.mybir as mybir
    from concourse.tile import TileContext

    i16 = mybir.dt.int16 if DTYPE == "i16" else mybir.dt.float16
    nc = bass.Bass(target_bir_lowering=False)

    sim = nc.dram_tensor("sim", [N_TILES, P, B], i16, kind="ExternalInput")
    # columns 0..B-1: per-column labels (same in every partition);
    # column B+t: labels of tile t's 128 rows. Values in [-2048, 2047] (exact).
    labs = nc.dram_tensor("labs", [P, B + N_TILES], i16, kind="ExternalInput")
    # one extra (garbage) column on each output: the out-DMA reads it, and a
    # DVE memset of it afterwards observes the out-DMA's semaphore (WAR) so
    # the kernel-tail drain can wait on the DVE semaphore alone.
    row_an = nc.dram_tensor("row_an", [P, N_TILES + 1], i16, kind="ExternalOutput")
    colmax = nc.dram_tensor("colmax", [P, B + 1], i16, kind="ExternalOutput")

    tpc = N_TILES // NCH

    with TileContext(nc) as tc:
        with tc.tile_pool(name="pp", bufs=1) as pp:
            lt = pp.tile([P, B + N_TILES], i16, tag="labs")
            an_t = pp.tile([P, N_TILES + 1], i16, tag="an")
            sa = pp.tile([P, N_TILES * B], i16, tag="simall")
            mk = pp.tile([P, B], i16, tag="mk")  # masked tile (DVE-private)
            acc = pp.tile([P, B + 1], i16, tag="acc")  # running col max
            absorb = pp.tile([P, 1], i16, tag="absorb")

            nc.sync.dma_start(out=lt[:], in_=labs[:])
            # Observe the labs-DMA semaphore on DVE before any real consumer.
            nc.vector.tensor_copy(absorb[:], lt[:, :1])

            for k in range(NCH):
                nc.sync.dma_start(
                    out=sa[:, k * tpc * B : (k + 1) * tpc * B].rearrange(
                        "p (t j) -> p t j", j=B
                    ),
                    in_=sim[k * tpc : (k + 1) * tpc].rearrange("t p j -> p t j"),
                )

            for t in range(N_TILES):
                raw = sa[:, t * B : (t + 1) * B]
                md = acc[:, :B] if t == 0 else mk[:]
                if t % tpc == 0:
                    # Observe this chunk's DMA semaphore on DVE: single-cell
                    # self-copy inside the chunk. Its only dependency is the
                    # chunk DMA; the chunk's consumers are ordered after it
                    # through the engine's own semaphore, which they already
                    # carry.
                    nc.vector.tensor_copy(raw[:, :1], raw[:, :1])
                # masked = (labcol != labrow) * sim
                # tile 0 writes the masked values straight into the
                # accumulator: saves one full copy pass
                nc.vector.scalar_tensor_tensor(
                    out=md,
                    in0=lt[:, :B],
                    scalar=lt[:, B + t : B + t + 1],
                    in1=raw,
                    op0=mybir.AluOpType.not_equal,
                    op1=mybir.AluOpType.mult,
                )
                nc.vector.tensor_reduce(
                    an_t[:, t : t + 1],
                    md,
                    mybir.AxisListType.X,
                    mybir.AluOpType.max,
                )
                if t > 0:
                    nc.vector.tensor_max(acc[:, :B], acc[:, :B], mk[:])

            # Output DMAs on the Activation HWDGE queue (fresh semaphores).
            nc.scalar.dma_start(out=row_an[:], in_=an_t[:])
            nc.scalar.dma_start(out=colmax[:], in_=acc[:])
            # Observe each out-DMA's semaphore on DVE by overwriting the
            # garbage column it read (pure WAR dependency: one wait each).
            nc.vector.memset(an_t[:, N_TILES:], 0)
            nc.vector.memset(acc[:, B:], 0)

    _fix_tail_drain(nc)
    return nc


def _fix_tail_drain(nc):
    """This walrus build encodes a single sync-wait per instruction, but the
    kernel-tail drain waits on every DMA semaphore plus the DVE semaphore.
    Every DMA semaphore is observed by a DVE instruction (absorber copies for
    loads, garbage-column memsets for stores), so the DVE-semaphore wait alone
    transitively implies all of them: drop the rest."""
    dma_sems = set()
    for ins in nc.inst_map.values():
        if type(ins).__name__ == "InstDMACopy":
            si = getattr(ins, "sync_info", None)
            for u in (getattr(si, "on_update", None) or []):
                dma_sems.add(u.id)
    for ins in nc.inst_map.values():
        if type(ins).__name__ == "InstDrain":
            si = getattr(ins, "sync_info", None)
            w = (getattr(si, "on_wait", None) or []) if si else []
            if len(w) > 1:
                keep = [x for x in w if x.id not in dma_sems]
                assert len(keep) == 1, [(x.id, x.wait_value) for x in w]
                si.on_wait = keep


def kernel(similarity, labels, margin, semi):
    global last_results
    from concourse.bass_utils import run_bass_kernel_spmd

    sim = np.asarray(similarity, dtype=np.float32)
    lab = np.asarray(labels).reshape(-1)
    marg = np.asarray(margin, dtype=np.float32).reshape(-1)

    # Dense-rank labels into [-2048, 2047] (exact in both int16 and fp16;
    # equality preserved).
    np_dt = np.int16 if DTYPE == "i16" else np.float16
    _, inv = np.unique(lab, return_inverse=True)
    lab16 = (inv.astype(np.int32) - 2048).astype(np_dt)
    labcols = np.broadcast_to(lab16[None, :], (P, B))

    # Fixed-point int16 encoding of the similarity matrix. Masking multiplies
    # by 0/1 and max-mining is order-preserving, so the mined values carry
    # only the +-1e-4 quantization of this rounding - no fp16 max-selection
    # bias. Host arithmetic stays f32 and the diagonal is exact.
    if DTYPE == "i16":
        sim16 = np.clip(np.rint(sim * SCALE), -32700, 32700).astype(np.int16)
    else:
        sim16 = sim.astype(np.float16)

    if "nc" not in _cache:
        _cache["nc"] = _build_bass()
    nc = _cache["nc"]

    in_maps = []
    for c in range(N_CORES):
        r0 = c * ROWS_PER_CORE
        shard = sim16[r0 : r0 + ROWS_PER_CORE].reshape(N_TILES, P, B)
        lr = lab16[r0 : r0 + ROWS_PER_CORE].reshape(N_TILES, P).T  # [P, N_TILES]
        labs = np.ascontiguousarray(
            np.concatenate([labcols, lr], axis=1, dtype=np_dt)
        )
        in_maps.append({"sim": shard, "labs": labs})

    trace = os.environ.get("CRL_TRACE", "0") == "1"
    res = run_bass_kernel_spmd(
        nc, in_maps, core_ids=list(range(N_CORES)), trace=trace
    )
    last_results = res

    # an for row r = c*1024 + t*128 + p  at row_an[p, t]; drop garbage column
    inv_s = np.float32(1.0 / SCALE)
    an_row = np.concatenate(
        [r["row_an"][:, :N_TILES].astype(np.float32).T.reshape(-1) for r in res.results]
    ) * inv_s  # [B]
    colp = np.stack([r["colmax"][:, :B] for r in res.results]).astype(np.float32)
    an_col = colp.reshape(N_CORES * P, B).max(axis=0) * inv_s  # [B]

    ap = np.ascontiguousarray(np.diagonal(sim))
    mam = marg - ap  # f32

    def one_side(an):
        valid = an > ap
        loss = np.maximum(mam + an, np.float32(0.0))
        return np.where(valid, loss, np.float32(0.0)).sum(dtype=np.float32)

    total = np.float32(one_side(an_row)) + np.float32(one_side(an_col))
    return np.asarray(total, dtype=np.float32)



# revision 11
# speedup vs baseline: 2.1736x; 2.1736x over previous
"""CRLoss (hard-negative triplet mining over a [B,B] similarity matrix) on 8 trn2 cores.

Sharding: rows of `similarity` split across 8 cores (1024 rows each; 8 row-tiles
of [128, 8192] fp16 per core).

Device computes UNMASKED per-row max (hardest value incl. same-label cols) and
per-partition column-max partials of the fp16 matrix. No labels on device: the
label mask only matters for the ~B/4096-per-row same-label columns, so the host
(a) computes each row/col's max over its own label group (tiny: sum of squared
group sizes ~ 3*B elements) and (b) for the handful of rows/cols where that
excluded max ties the device's unmasked max, recomputes the masked max exactly
from the fp16 matrix. All loss arithmetic stays f32 on host with the exact f32
diagonal.

Why fp16 (not int16 as before): the DVE's 2x_1p perf mode only engages for
16-bit float dtypes, halving tensor_tensor cycles. Row maxes per tile use a
tensor_max fold tree at 2x (8192 -> 512) plus one 1x tensor_reduce on the 512
remainder (~4.6k cycles vs 8.25k for a plain reduce; InstTensorTensorReduce
and the custom-DVE ISA ops do not codegen in this walrus build - "ISA wrong
length"). Column partials use tensor_max chains at 2x, with the last tile done
in four column quarters so its results stream out over two DMA rings while the
final row-reduce still runs.

Sync discipline (this compiler build encodes ONE sync-wait per instruction):
an "absorber" 1-cell self-copy observes each chunk-DMA semaphore on DVE before
the chunk's first real consumer; output DMAs are observed by DVE memsets of a
garbage column each output DMA also reads (pure WAR); the kernel-tail drain
then only needs the DVE semaphore (_fix_tail_drain strips the rest).
"""

import os

import numpy as np

B = 8192
N_CORES = 8
ROWS_PER_CORE = B // N_CORES  # 1024
P = 128
N_TILES = ROWS_PER_CORE // P  # 8
H = B // 2  # 4096
Q = B // 4  # 2048
F = 512  # row fold-tree stops here; tensor_reduce finishes

_cache: dict = {}
last_results = None  # BassKernelResults from the most recent run (for test.py)


def _build_bass():
    import concourse.bass as bass
    import concourse.mybir as mybir
    from concourse.tile import TileContext

    f16 = mybir.dt.float16
    Alu = mybir.AluOpType
    nc = bass.Bass(target_bir_lowering=False)

    sim = nc.dram_tensor("sim", [N_TILES, P, B], f16, kind="ExternalInput")
    an = nc.dram_tensor("an", [P, N_TILES], f16, kind="ExternalOutput")
    cms = [
        nc.dram_tensor(f"cm{q}", [P, H], f16, kind="ExternalOutput")
        for q in range(2)
    ]
    # The Tile scheduler has 8 DMA-completion semaphore lanes; a 9th DMA
    # reuses a lane and needs a second sync-wait, which this walrus build
    # rejects. Keep total DMA count at 8: 5 loads + 2 colmax halves + an.
    CHUNKS = [(0, 1), (1, 2), (2, 4), (4, 6), (6, 8)]

    with TileContext(nc) as tc:
        with tc.tile_pool(name="pp", bufs=1) as pp:
            sa = pp.tile([P, N_TILES * B], f16, tag="simall")
            acc = pp.tile([P, B], f16, tag="acc")
            an_t = pp.tile([P, N_TILES], f16, tag="an")
            fold = pp.tile([P, H], f16, tag="fold")

            for lo_t, hi_t in CHUNKS:
                nc.sync.dma_start(
                    out=sa[:, lo_t * B : hi_t * B].rearrange(
                        "p (t j) -> p t j", j=B
                    ),
                    in_=sim[lo_t:hi_t].rearrange("t p j -> p t j"),
                )

            def row_max(t):
                # tensor_max fold tree at 2x: 8192 -> 512, then one 1x
                # tensor_reduce over the 512 remainder.
                raw = sa[:, t * B : (t + 1) * B]
                nc.vector.tensor_max(fold[:, :H], raw[:, :H], raw[:, H:])
                w = H // 2
                while w >= F:
                    nc.vector.tensor_max(fold[:, :w], fold[:, :w], fold[:, w : 2 * w])
                    w //= 2
                nc.vector.tensor_reduce(
                    an_t[:, t : t + 1],
                    fold[:, : 2 * F][:, :F],
                    mybir.AxisListType.X,
                    Alu.max,
                )

            chunk_first = {lo_t for lo_t, _ in CHUNKS}
            last = N_TILES - 1
            for t in range(N_TILES):
                raw = sa[:, t * B : (t + 1) * B]
                if t in chunk_first:
                    # Absorber: observe the chunk's DMA semaphore on DVE so
                    # real consumers only carry the DVE-semaphore wait.
                    nc.vector.tensor_copy(raw[:, :1], raw[:, :1])
                if t != last:
                    row_max(t)
                    if t == 1:
                        nc.vector.tensor_max(acc[:], sa[:, :B], raw)
                    elif t >= 2:
                        nc.vector.tensor_max(acc[:], acc[:], raw)
                else:
                    # Last tile: column halves first so their out-DMAs
                    # stream over two rings while the final row-reduce runs.
                    for q in range(2):
                        lo, hi = q * H, (q + 1) * H
                        nc.vector.tensor_max(
                            acc[:, lo:hi], acc[:, lo:hi], raw[:, lo:hi]
                        )
                        ring = nc.scalar if q == 0 else nc.sync
                        ring.dma_start(out=cms[q][:], in_=acc[:, lo:hi])
                    row_max(t)
                    nc.scalar.dma_start(out=an[:], in_=an_t[:])

            # Observe each out-DMA's semaphore on DVE by overwriting one cell
            # the DMA read (pure WAR dependency: one wait each).
            for q in range(2):
                nc.vector.memset(acc[:, q * H : q * H + 1], 0)
            nc.vector.memset(an_t[:, :1], 0)

    _fix_tail_drain(nc)
    return nc


def _fix_tail_drain(nc):
    """This walrus build encodes a single sync-wait per instruction, but the
    kernel-tail drain waits on every DMA semaphore plus the DVE semaphore.
    Every DMA semaphore is observed by a DVE instruction (absorber copies for
    loads, garbage-column memsets for stores), so the DVE-semaphore wait alone
    transitively implies all of them: drop the rest."""
    dma_sems = set()
    for ins in nc.inst_map.values():
        if type(ins).__name__ == "InstDMACopy":
            si = getattr(ins, "sync_info", None)
            for u in (getattr(si, "on_update", None) or []):
                dma_sems.add(u.id)
    for ins in nc.inst_map.values():
        if type(ins).__name__ == "InstDrain":
            si = getattr(ins, "sync_info", None)
            w = (getattr(si, "on_wait", None) or []) if si else []
            if len(w) > 1:
                keep = [x for x in w if x.id not in dma_sems]
                assert len(keep) == 1, [(x.id, x.wait_value) for x in w]
                si.on_wait = keep


def _label_group_maxes(sim16f, lab):
    """For every row i: max over columns with the same label (incl. diagonal);
    for every column j: max over rows with the same label. O(sum |group|^2)."""
    erow = np.full(B, -np.inf, dtype=np.float32)
    ecol = np.full(B, -np.inf, dtype=np.float32)
    order = np.argsort(lab, kind="stable")
    sl = lab[order]
    starts = np.flatnonzero(np.r_[True, sl[1:] != sl[:-1]])
    bounds = np.r_[starts, len(sl)]
    groups = []
    for k in range(len(starts)):
        M = order[bounds[k] : bounds[k + 1]]
        sub = sim16f[np.ix_(M, M)]
        erow[M] = sub.max(axis=1)
        ecol[M] = sub.max(axis=0)
        groups.append(M)
    # member list per row index
    members = {}
    for M in groups:
        for i in M:
            members[int(i)] = M
    return erow, ecol, members


def kernel(similarity, labels, margin, semi):
    global last_results
    from concourse.bass_utils import run_bass_kernel_spmd

    sim = np.asarray(similarity, dtype=np.float32)
    lab = np.asarray(labels).reshape(-1)
    marg = np.asarray(margin, dtype=np.float32).reshape(-1)

    sim16 = sim.astype(np.float16)

    if "nc" not in _cache:
        _cache["nc"] = _build_bass()
    nc = _cache["nc"]

    in_maps = []
    for c in range(N_CORES):
        r0 = c * ROWS_PER_CORE
        in_maps.append(
            {"sim": sim16[r0 : r0 + ROWS_PER_CORE].reshape(N_TILES, P, B)}
        )

    trace = os.environ.get("CRL_TRACE", "0") == "1"
    res = run_bass_kernel_spmd(
        nc, in_maps, core_ids=list(range(N_CORES)), trace=trace
    )
    last_results = res

    # Device unmasked maxes. Row r = c*1024 + t*128 + p lives at an[p, t].
    rmax = np.concatenate(
        [r["an"][:, :N_TILES].astype(np.float32).T.reshape(-1) for r in res.results]
    )  # [B]
    cmax_parts = []
    for q in range(2):
        part = np.stack([r[f"cm{q}"] for r in res.results])  # [8,128,H]
        cmax_parts.append(part.astype(np.float32).max(axis=(0, 1)))
    cmax = np.concatenate(cmax_parts)  # [B]

    # Host-side label-mask fixup.
    sim16f = sim16.astype(np.float32)
    erow, ecol, members = _label_group_maxes(sim16f, lab)

    an_row = rmax.copy()
    for i in np.flatnonzero(erow >= rmax):
        r = sim16f[i].copy()
        r[members[int(i)]] = -np.inf
        an_row[i] = r.max()
    an_col = cmax.copy()
    for j in np.flatnonzero(ecol >= cmax):
        c = sim16f[:, j].copy()
        c[members[int(j)]] = -np.inf
        an_col[j] = c.max()

    ap = np.ascontiguousarray(np.diagonal(sim))  # exact f32
    mam = marg - ap

    def one_side(an):
        valid = an > ap
        loss = np.maximum(mam + an, np.float32(0.0))
        return np.where(valid, loss, np.float32(0.0)).sum(dtype=np.float32)

    total = np.float32(one_side(an_row)) + np.float32(one_side(an_col))
    return np.asarray(total, dtype=np.float32)
